# revision 6
# baseline (speedup 1.0000x reference)
"""Trainium2 Bass kernel for nn_MDlayer (min-plus "tropical" layer).

    out[b,u] = min( min_d(x[b,d] - Wmin[u,d]), min_d(Wmax[u,d] - x[b,d]) )
             = min_d min(x[b,d] - Wmin[u,d], Wmax[u,d] - x[b,d])

Shapes: x [256, 1024], Wmin/Wmax [512, 1024] -> out [256, 512], fp32.

Strategy (8 NeuronCores, tensor-parallel over units U):
  - Each core owns U_LOC = 64 units; every core sees the full x.
  - Layout on-chip: partitions = 128 d's (8 d-groups), free dim = all 256 b's.
  - One custom DVE op per (u, d-group) computes
        acc[d,b] = min(min(x[d,b] - Wmin[u,d], Wmax[u,d] - x[d,b]), acc[d,b])
    in a single 1-elem/cycle pass (Wmin/Wmax columns ride the per-partition
    scalar slots, so x is the only streamed input - 1 DVE cycle per (b,u,d)
    element, the engine floor for a min-reduction that TensorE cannot do).
  - The remaining min over the 128 resident d's: PE transpose (128x128) to
    PSUM, then a DVE tensor_reduce(min) over the free dim.
  - Host gathers per-core [256, 64] slabs -> [256, 512].
"""

from contextlib import ExitStack

import numpy as np

from concourse import bacc, mybir, tile
from concourse import dve_ops
from concourse.dve_spec import Spec, Src0, Src1, C0, C1, minn, lower, _has_src1
from concourse.dve_uop import DveOpSpec
from concourse.bass_utils import run_bass_kernel_spmd

N_CORES = 8
B, U, D = 256, 512, 1024
U_LOC = U // N_CORES          # 64 units per core
NG = D // 128                 # 8 d-groups of 128 partitions
F32 = mybir.dt.float32


def _register_op(name: str, spec: Spec, subdim: bool = False):
    """Register a new custom DVE op at runtime (row 17+, sha computed here)."""
    for o in dve_ops.OPS:
        if o.name == name:
            return o
    row = dve_ops._CUSTOM_DVE_ROW_BASE + len(dve_ops.OPS)
    dve_ops._SUB_OPCODE_FOR_NAME[name] = row
    shas = {}
    for ver in ("v3", "v4"):
        uops = lower(spec, ver=ver)
        shas[ver] = DveOpSpec(
            name=name, opcode=row, uops=uops, rd1_en=_has_src1(spec)
        ).sha(ver)
    op = dve_ops.DveOp(name, spec, subdim, uops_sha=shas)
    dve_ops.OPS.append(op)
    dve_ops.CUSTOM_DVE_SPECS[name] = spec
    return op


# acc_new = min(x - wmin, wmax - x)            (first d-group of a unit)
MD_MIN2 = _register_op(
    "MD_MIN2_ANT",
    Spec(
        body=minn(Src0 - C0, C1 - Src0),
        reference=lambda in0, in1, s0, s1, imm2: np.minimum(
            in0.astype(np.float32) - s0, s1 - in0.astype(np.float32)
        ).astype(np.float32),
    ),
)

# acc_new = min(min(x - wmin, wmax - x), acc)  (remaining d-groups, RMW)
MD_MIN2_ACC = _register_op(
    "MD_MIN2_ACC_ANT",
    Spec(
        body=minn(minn(Src0 - C0, C1 - Src0), Src1),
        reference=lambda in0, in1, s0, s1, imm2: np.minimum(
            np.minimum(in0.astype(np.float32) - s0, s1 - in0.astype(np.float32)),
            in1,
        ).astype(np.float32),
    ),
)

_NC_CACHE = {}


def _build():
    """Build + compile the per-core Bass module (identical on all cores)."""
    if "nc" in _NC_CACHE:
        return _NC_CACHE["nc"]
    nc = bacc.Bacc(
        "TRN2", target_bir_lowering=False, debug=False, num_devices=N_CORES
    )
    # Host pre-packs everything into [128, ...] slab-major layouts so each
    # input is one dense DMA.
    xT = nc.dram_tensor("xT", [128, NG * B], F32, kind="ExternalInput").ap()
    wminT = nc.dram_tensor("wminT", [128, NG * U_LOC], F32, kind="ExternalInput").ap()
    wmaxT = nc.dram_tensor("wmaxT", [128, NG * U_LOC], F32, kind="ExternalInput").ap()
    ident = nc.dram_tensor("ident", [128, 128], F32, kind="ExternalInput").ap()
    out = nc.dram_tensor("out", [B, U_LOC], F32, kind="ExternalOutput").ap()

    QUAD = 8  # u's processed in lock-step: breaks RAW chains, batches reduces

    with tile.TileContext(nc) as tc, ExitStack() as ctx:
        const_pool = ctx.enter_context(tc.tile_pool(name="const", bufs=1))
        acc_pool = ctx.enter_context(tc.tile_pool(name="acc", bufs=2 * QUAD + 2))
        psum_pool = ctx.enter_context(tc.tile_pool(name="ps", bufs=2, space="PSUM"))
        ob_pool = ctx.enter_context(tc.tile_pool(name="ob", bufs=3))

        # Small weight tiles first so compute can start ~immediately.
        wmin_sb = const_pool.tile([128, NG * U_LOC], F32, tag="wmin")
        nc.sync.dma_start(wmin_sb[:], wminT[:])
        wmax_sb = const_pool.tile([128, NG * U_LOC], F32, tag="wmax")
        nc.sync.dma_start(wmax_sb[:], wmaxT[:])
        id_sb = const_pool.tile([128, 128], F32, tag="ident")
        nc.sync.dma_start(id_sb[:], ident[:])
        # x: one DMA per d-group so group 0 compute overlaps later loads.
        xt_sb = const_pool.tile([128, NG * B], F32, tag="xt")
        for g in range(NG):
            nc.sync.dma_start(xt_sb[:, g * B:(g + 1) * B], xT[:, g * B:(g + 1) * B])
        for q in range(U_LOC // QUAD):
            us = [q * QUAD + i for i in range(QUAD)]
            accs = [acc_pool.tile([128, B], F32, name=f"acc_u{u}", tag="acc") for u in us]
            for g in range(NG):
                in0 = xt_sb[:, g * B:(g + 1) * B]
                for i, u in enumerate(us):
                    s0 = wmin_sb[:, g * U_LOC + u: g * U_LOC + u + 1]
                    s1 = wmax_sb[:, g * U_LOC + u: g * U_LOC + u + 1]
                    if g == 0:
                        nc.vector._custom_dve(
                            MD_MIN2, out=accs[i][:], in0=in0, s0=s0, s1=s1
                        )
                    else:
                        nc.vector._custom_dve(
                            MD_MIN2_ACC, out=accs[i][:], in0=in0, in1=accs[i][:],
                            s0=s0, s1=s1,
                        )
            # Transpose all QUAD accs (both b-halves) into one 4-bank PSUM tile,
            # one batched min-reduce yields QUAD output columns for both halves,
            # then stream this quad's slab of the output to DRAM.
            ps = psum_pool.tile([128, 2, QUAD, 128], F32)
            for s in range(2):
                for i in range(QUAD):
                    nc.tensor.transpose(
                        ps[:, s, i, :], accs[i][:, s * 128:(s + 1) * 128], id_sb[:]
                    )
            ob = ob_pool.tile([128, 2, QUAD], F32, tag="ob")
            nc.vector.tensor_reduce(
                ob[:], ps[:], mybir.AxisListType.X, mybir.AluOpType.min
            )
            nc.sync.dma_start(out[0:128, q * QUAD:(q + 1) * QUAD], ob[:, 0, :])
            nc.sync.dma_start(out[128:256, q * QUAD:(q + 1) * QUAD], ob[:, 1, :])
    nc.compile()
    _NC_CACHE["nc"] = nc
    return nc


def _pack_dT(a: np.ndarray, ncols: int) -> np.ndarray:
    """[R, 1024] -> transposed slab-major [128, 8*R]: out[p, g*R + r] = a[r, g*128+p]."""
    return np.ascontiguousarray(
        a.T.reshape(NG, 128, ncols).transpose(1, 0, 2).reshape(128, NG * ncols)
    )


def kernel(x: np.ndarray, Wmin: np.ndarray, Wmax: np.ndarray) -> np.ndarray:
    assert x.shape == (B, D) and Wmin.shape == (U, D) and Wmax.shape == (U, D)
    nc = _build()
    x = np.ascontiguousarray(x, dtype=np.float32)
    xT = _pack_dT(x, B)
    ident = np.eye(128, dtype=np.float32)
    in_maps = []
    for c in range(N_CORES):
        us = slice(c * U_LOC, (c + 1) * U_LOC)
        in_maps.append({
            "xT": xT,
            "wminT": _pack_dT(np.ascontiguousarray(Wmin[us], dtype=np.float32), U_LOC),
            "wmaxT": _pack_dT(np.ascontiguousarray(Wmax[us], dtype=np.float32), U_LOC),
            "ident": ident,
        })
    res = run_bass_kernel_spmd(nc, in_maps, list(range(N_CORES)))
    return np.concatenate([res.results[c]["out"] for c in range(N_CORES)], axis=1)


# revision 7
# speedup vs baseline: 1.0058x; 1.0058x over previous
"""Trainium2 Bass kernel for nn_MDlayer (min-plus "tropical" layer).

    out[b,u] = min( min_d(x[b,d] - Wmin[u,d]), min_d(Wmax[u,d] - x[b,d]) )
             = min_d min(x[b,d] - Wmin[u,d], Wmax[u,d] - x[b,d])

Shapes: x [256, 1024], Wmin/Wmax [512, 1024] -> out [256, 512], fp32.

Strategy (8 NeuronCores, tensor-parallel over units U):
  - Each core owns U_LOC = 64 units; every core sees the full x.
  - Layout on-chip: partitions = 128 d's (8 d-groups), free dim = all 256 b's.
  - One custom DVE op per (u, d-group) computes
        acc[d,b] = min(min(x[d,b] - Wmin[u,d], Wmax[u,d] - x[d,b]), acc[d,b])
    in a single 1-elem/cycle pass (Wmin/Wmax columns ride the per-partition
    scalar slots, so x is the only streamed input - 1 DVE cycle per (b,u,d)
    element, the engine floor for a min-reduction that TensorE cannot do).
  - The remaining min over the 128 resident d's: PE transpose (128x128) to
    PSUM, then a DVE tensor_reduce(min) over the free dim.
  - Host gathers per-core [256, 64] slabs -> [256, 512].
"""

from contextlib import ExitStack

import numpy as np

from concourse import bacc, mybir, tile
from concourse import dve_ops
from concourse.dve_spec import Spec, Src0, Src1, C0, C1, minn, lower, _has_src1
from concourse.dve_uop import DveOpSpec
from concourse.bass_utils import run_bass_kernel_spmd

N_CORES = 8
B, U, D = 256, 512, 1024
U_LOC = U // N_CORES          # 64 units per core
NG = D // 128                 # 8 d-groups of 128 partitions
F32 = mybir.dt.float32


def _register_op(name: str, spec: Spec, subdim: bool = False):
    """Register a new custom DVE op at runtime (row 17+, sha computed here)."""
    for o in dve_ops.OPS:
        if o.name == name:
            return o
    row = dve_ops._CUSTOM_DVE_ROW_BASE + len(dve_ops.OPS)
    dve_ops._SUB_OPCODE_FOR_NAME[name] = row
    shas = {}
    for ver in ("v3", "v4"):
        uops = lower(spec, ver=ver)
        shas[ver] = DveOpSpec(
            name=name, opcode=row, uops=uops, rd1_en=_has_src1(spec)
        ).sha(ver)
    op = dve_ops.DveOp(name, spec, subdim, uops_sha=shas)
    dve_ops.OPS.append(op)
    dve_ops.CUSTOM_DVE_SPECS[name] = spec
    return op


# acc_new = min(x - wmin, wmax - x)            (first d-group of a unit)
MD_MIN2 = _register_op(
    "MD_MIN2_ANT",
    Spec(
        body=minn(Src0 - C0, C1 - Src0),
        reference=lambda in0, in1, s0, s1, imm2: np.minimum(
            in0.astype(np.float32) - s0, s1 - in0.astype(np.float32)
        ).astype(np.float32),
    ),
)

# acc_new = min(min(x - wmin, wmax - x), acc)  (remaining d-groups, RMW)
MD_MIN2_ACC = _register_op(
    "MD_MIN2_ACC_ANT",
    Spec(
        body=minn(minn(Src0 - C0, C1 - Src0), Src1),
        reference=lambda in0, in1, s0, s1, imm2: np.minimum(
            np.minimum(in0.astype(np.float32) - s0, s1 - in0.astype(np.float32)),
            in1,
        ).astype(np.float32),
    ),
)

_NC_CACHE = {}


def _build():
    """Build + compile the per-core Bass module (identical on all cores)."""
    if "nc" in _NC_CACHE:
        return _NC_CACHE["nc"]
    nc = bacc.Bacc(
        "TRN2", target_bir_lowering=False, debug=False, num_devices=N_CORES
    )
    # Host pre-packs everything into [128, ...] slab-major layouts so each
    # input is one dense DMA.
    xT = nc.dram_tensor("xT", [128, NG * B], F32, kind="ExternalInput").ap()
    wminT = nc.dram_tensor("wminT", [128, NG * U_LOC], F32, kind="ExternalInput").ap()
    wmaxT = nc.dram_tensor("wmaxT", [128, NG * U_LOC], F32, kind="ExternalInput").ap()
    ident = nc.dram_tensor("ident", [128, 128], F32, kind="ExternalInput").ap()
    out = nc.dram_tensor("out", [B, U_LOC], F32, kind="ExternalOutput").ap()

    QUAD = 4  # u's processed in lock-step: breaks RAW chains, batches reduces

    with tile.TileContext(nc) as tc, ExitStack() as ctx:
        const_pool = ctx.enter_context(tc.tile_pool(name="const", bufs=1))
        acc_pool = ctx.enter_context(tc.tile_pool(name="acc", bufs=2 * QUAD + 2))
        psum_pool = ctx.enter_context(tc.tile_pool(name="ps", bufs=4, space="PSUM"))
        ob_pool = ctx.enter_context(tc.tile_pool(name="ob", bufs=3))

        # Small weight tiles first so compute can start ~immediately.
        wmin_sb = const_pool.tile([128, NG * U_LOC], F32, tag="wmin")
        nc.sync.dma_start(wmin_sb[:], wminT[:])
        wmax_sb = const_pool.tile([128, NG * U_LOC], F32, tag="wmax")
        nc.sync.dma_start(wmax_sb[:], wmaxT[:])
        id_sb = const_pool.tile([128, 128], F32, tag="ident")
        nc.sync.dma_start(id_sb[:], ident[:])
        # x: one DMA per d-group so group 0 compute overlaps later loads.
        xt_sb = const_pool.tile([128, NG * B], F32, tag="xt")
        for g in range(NG):
            nc.sync.dma_start(xt_sb[:, g * B:(g + 1) * B], xT[:, g * B:(g + 1) * B])
        for q in range(U_LOC // QUAD):
            us = [q * QUAD + i for i in range(QUAD)]
            accs = [acc_pool.tile([128, B], F32, name=f"acc_u{u}", tag="acc") for u in us]
            for g in range(NG):
                in0 = xt_sb[:, g * B:(g + 1) * B]
                for i, u in enumerate(us):
                    s0 = wmin_sb[:, g * U_LOC + u: g * U_LOC + u + 1]
                    s1 = wmax_sb[:, g * U_LOC + u: g * U_LOC + u + 1]
                    if g == 0:
                        nc.vector._custom_dve(
                            MD_MIN2, out=accs[i][:], in0=in0, s0=s0, s1=s1
                        )
                    else:
                        nc.vector._custom_dve(
                            MD_MIN2_ACC, out=accs[i][:], in0=in0, in1=accs[i][:],
                            s0=s0, s1=s1,
                        )
            # Transpose all QUAD accs (both b-halves) into one 2-bank PSUM tile,
            # one batched min-reduce yields QUAD output columns for both halves,
            # then stream this quad's slab of the output to DRAM.
            ps = psum_pool.tile([128, 2, QUAD, 128], F32)
            for s in range(2):
                for i in range(QUAD):
                    nc.tensor.transpose(
                        ps[:, s, i, :], accs[i][:, s * 128:(s + 1) * 128], id_sb[:]
                    )
            ob = ob_pool.tile([128, 2, QUAD], F32, tag="ob")
            nc.vector.tensor_reduce(
                ob[:], ps[:], mybir.AxisListType.X, mybir.AluOpType.min
            )
            nc.sync.dma_start(out[0:128, q * QUAD:(q + 1) * QUAD], ob[:, 0, :])
            nc.sync.dma_start(out[128:256, q * QUAD:(q + 1) * QUAD], ob[:, 1, :])
    nc.compile()
    _NC_CACHE["nc"] = nc
    return nc


def _pack_dT(a: np.ndarray, ncols: int) -> np.ndarray:
    """[R, 1024] -> transposed slab-major [128, 8*R]: out[p, g*R + r] = a[r, g*128+p]."""
    return np.ascontiguousarray(
        a.T.reshape(NG, 128, ncols).transpose(1, 0, 2).reshape(128, NG * ncols)
    )


def kernel(x: np.ndarray, Wmin: np.ndarray, Wmax: np.ndarray) -> np.ndarray:
    assert x.shape == (B, D) and Wmin.shape == (U, D) and Wmax.shape == (U, D)
    nc = _build()
    x = np.ascontiguousarray(x, dtype=np.float32)
    xT = _pack_dT(x, B)
    ident = np.eye(128, dtype=np.float32)
    in_maps = []
    for c in range(N_CORES):
        us = slice(c * U_LOC, (c + 1) * U_LOC)
        in_maps.append({
            "xT": xT,
            "wminT": _pack_dT(np.ascontiguousarray(Wmin[us], dtype=np.float32), U_LOC),
            "wmaxT": _pack_dT(np.ascontiguousarray(Wmax[us], dtype=np.float32), U_LOC),
            "ident": ident,
        })
    res = run_bass_kernel_spmd(nc, in_maps, list(range(N_CORES)))
    return np.concatenate([res.results[c]["out"] for c in range(N_CORES)], axis=1)
